# revision 27
# baseline (speedup 1.0000x reference)
"""Trainium2 Bass kernel for nn_FixedLatentNoiseDecoder.

Strategy (pure data-parallel, 1 batch sample per NeuronCore, no collectives):

  lats branch:  lats[b,t] = (env[b,t] - mean_T env[b]) @ W_blockdiag
     - mean over T commutes with the linear map, so the input is centered on
       the host (tiny: 3 MB of x) and shipped pre-transposed [36, T].
     - W_blockdiag [36, 18*512] is built on the host from latents_table and
       replicated; the device does K=36 fp32 matmuls per 128-token tile.

  noise branch: out_i = blur(mu + sig*eps_i) along T (9-tap, per-sample sigma)
     - blur is linear: out = Band @ (sig .* eps) + (Band @ mu).
     - Band is a banded Toeplitz matrix built on the host from the 9 Gaussian
       taps (sigma derives from x only), with reflect-boundary variants for
       the first/last 128-token tile.  Per out-tile the contraction over
       128+8 input tokens is done as 3 PSUM-accumulated matmuls:
       prev-halo (K=4), current (K=128), next-halo (K=4).
     - sig is folded in by row-scaling the eps tile on the vector engine;
       Band@mu ("blurmu", derives from x only) is a host scalar per token,
       folded into the PSUM->SBUF copy as a per-partition bias on ScalarE.

All heavy I/O (86 MB of stores, 11 MB of eps loads per core) is large
contiguous DMA.
"""

import numpy as np

import concourse.bass as bass
import concourse.bacc as bacc
import concourse.mybir as mybir
import concourse.tile as tile
from concourse.bass_utils import run_bass_kernel_spmd

B, T = 8, 2048
S, H, WS, L = 3, 12, 6, 512
NENV = S * H          # 36
NLAT = S * WS * L     # 9216
R = 4
K9 = 2 * R + 1
N_NOISE = 4
SS = [16, 64, 256, 1024]   # s*s per noise level
PT = 128                   # tokens per tile
NTILE = T // PT            # 16
F32 = mybir.dt.float32
F16 = mybir.dt.float16     # matmul operand dtype: full PE rate + FWL weight loads

_NC_CACHE = None


def _build_nc():
    nc = bacc.Bacc("TRN2", target_bir_lowering=False, debug=False, num_devices=8)

    xct = nc.dram_tensor("xct", [NENV, T], F16, kind="ExternalInput")
    tbd = nc.dram_tensor("tbd", [NENV, NLAT], F16, kind="ExternalInput")
    xn = nc.dram_tensor("xn", [T, 12], F32, kind="ExternalInput")
    bandc = nc.dram_tensor("bandc", [3 * N_NOISE, PT, PT], F16, kind="ExternalInput")
    bande = nc.dram_tensor("bande", [N_NOISE, 2 * R, PT], F16, kind="ExternalInput")
    bmu = nc.dram_tensor("bmu", [T, N_NOISE], F32, kind="ExternalInput")
    eps = [nc.dram_tensor(f"eps{i}", [T, SS[i]], F32, kind="ExternalInput")
           for i in range(N_NOISE)]

    lats = nc.dram_tensor("lats", [T, NLAT], F32, kind="ExternalOutput")
    nout = [nc.dram_tensor(f"n{i}", [T, SS[i]], F32, kind="ExternalOutput")
            for i in range(N_NOISE)]

    with tile.TileContext(nc) as tc:
        with (
            tc.tile_pool(name="const", bufs=1) as cp,
            tc.tile_pool(name="lats_out", bufs=2) as lop,
            tc.tile_pool(name="eps0", bufs=8) as ep0,
            tc.tile_pool(name="eps1", bufs=8) as ep1,
            tc.tile_pool(name="eps2", bufs=8) as ep2,
            tc.tile_pool(name="eps3", bufs=8) as ep3,
            tc.tile_pool(name="epsraw", bufs=6) as epr,
            tc.tile_pool(name="nout0", bufs=2) as np0,
            tc.tile_pool(name="nout1", bufs=2) as np1,
            tc.tile_pool(name="nout2", bufs=2) as np2,
            tc.tile_pool(name="nout3", bufs=2) as np3,
            tc.tile_pool(name="tails", bufs=3) as tlp,
            tc.tile_pool(name="lpsum", bufs=3, space="PSUM") as lps,
            tc.tile_pool(name="npsum", bufs=2, space="PSUM") as nps,
        ):
            epools = [ep0, ep1, ep2, ep3]
            opools = [np0, np1, np2, np3]

            # --- constants, loaded once ---
            tb = cp.tile([NENV, NLAT], F16)
            nc.sync.dma_start(tb[:], tbd[:])
            xs = cp.tile([NENV, T], F16)
            nc.sync.dma_start(xs[:], xct[:])
            bc = cp.tile([PT, 3 * N_NOISE * PT], F16)
            for v in range(3 * N_NOISE):
                nc.gpsimd.dma_start(bc[:, v * PT:(v + 1) * PT], bandc[v])
            be = cp.tile([2 * R, N_NOISE * PT], F16)
            for v in range(N_NOISE):
                nc.gpsimd.dma_start(be[:, v * PT:(v + 1) * PT], bande[v])
            bm = cp.tile([PT, NTILE * N_NOISE], F32)
            xns = cp.tile([PT, NTILE * 12], F32)
            for n in range(NTILE):
                nc.gpsimd.dma_start(bm[:, n * N_NOISE:(n + 1) * N_NOISE],
                                  bmu[n * PT:(n + 1) * PT, :])
                nc.gpsimd.dma_start(xns[:, n * 12:(n + 1) * 12],
                                  xn[n * PT:(n + 1) * PT, :])

            # scaled-eps tiles, kept alive for the +/-1 tile halo
            seps = [[None] * NTILE for _ in range(N_NOISE)]

            def load_scale(i, n):
                s2 = SS[i]
                raw = epr.tile([PT, s2], F32, tag=f"raw{i}")
                nc.scalar.dma_start(raw[:], eps[i][n * PT:(n + 1) * PT, :])
                t_ = epools[i].tile([PT, s2], F16)
                sig = xns[:, n * 12 + 3 * i + 1: n * 12 + 3 * i + 2]
                nc.vector.tensor_scalar_mul(t_[:], raw[:], sig)
                seps[i][n] = t_

            for i in range(N_NOISE):
                for n0 in range(4):
                    load_scale(i, n0)

            for n in range(NTILE):
                for i in range(N_NOISE):
                    if n + 4 < NTILE:
                        load_scale(i, n + 4)

                # --- noise branch for tile n ---
                v = 0 if n == 0 else (2 if n == NTILE - 1 else 1)
                for i in range(N_NOISE):
                    s2 = SS[i]
                    hl = tlp.tile([2 * R, s2], F16, tag=f"halo{i}")
                    if n == 0 or n == NTILE - 1:
                        nc.gpsimd.memset(hl[:], 0.0)
                    if n > 0:
                        nc.gpsimd.dma_start(hl[0:R, :],
                                            seps[i][n - 1][PT - R:PT, :])
                    if n + 1 < NTILE:
                        nc.gpsimd.dma_start(hl[R:2 * R, :], seps[i][n + 1][0:R, :])
                    no = opools[i].tile([PT, s2], F32)
                    for c0 in range(0, s2, L):
                        cw = min(L, s2 - c0)
                        pq = nps.tile([PT, cw], F32)
                        nc.tensor.matmul(pq[:],
                                         bc[:, (3 * i + v) * PT:(3 * i + v + 1) * PT],
                                         seps[i][n][:, c0:c0 + cw],
                                         start=True, stop=False)
                        nc.tensor.matmul(pq[:], be[:, i * PT:(i + 1) * PT],
                                         hl[:, c0:c0 + cw],
                                         start=False, stop=True)
                        nc.scalar.activation(
                            no[:, c0:c0 + cw], pq[:],
                            mybir.ActivationFunctionType.Identity,
                            bias=bm[:, n * N_NOISE + i: n * N_NOISE + i + 1],
                            scale=1.0)
                    nc.sync.dma_start(nout[i][n * PT:(n + 1) * PT, :], no[:])

                # --- lats branch for tile n ---
                # two matmuls share a 2-bank PSUM tile; one wide copy drains both
                lo = lop.tile([PT, NLAT], F32)
                xt = xs[:, n * PT:(n + 1) * PT]
                for c2 in range(9):
                    pp = lps.tile([PT, 2 * L], F32)
                    for h in range(2):
                        c = 2 * c2 + h
                        nc.tensor.matmul(pp[:, h * L:(h + 1) * L], xt,
                                         tb[:, c * L:(c + 1) * L],
                                         start=True, stop=True)
                    if c2 % 3 == 2:
                        nc.scalar.copy(lo[:, c2 * 2 * L:(c2 + 1) * 2 * L], pp[:])
                    else:
                        nc.vector.tensor_copy(lo[:, c2 * 2 * L:(c2 + 1) * 2 * L],
                                              pp[:])
                parts = 4 if n < 2 else 2
                step = NLAT // parts
                for p_ in range(parts):
                    ring = nc.sync if p_ % 2 == 0 else nc.scalar
                    ring.dma_start(
                        lats[n * PT:(n + 1) * PT, p_ * step:(p_ + 1) * step],
                        lo[:, p_ * step:(p_ + 1) * step])

    nc.compile()
    return nc


def _band_blocks(w):
    """9-tap blur as banded-matrix blocks.  Returns (cur[3,128,128], edge[2,4,128]).

    Block convention: out[t] = sum_k Band[k, t] * n[k]; lhsT layout is
    [K(in-token), M(out-token)].  cur variants: 0=first tile (reflect),
    1=interior, 2=last tile (reflect).  edge: 0=prev halo, 1=next halo.
    """
    cur = np.zeros((PT, PT), np.float32)
    for off in range(-R, R + 1):
        cur += w[off + R] * np.eye(PT, PT, -off, dtype=np.float32)
    prev = np.zeros((R, PT), np.float32)
    nxt = np.zeros((R, PT), np.float32)
    for q in range(R):
        for off in range(-R, R + 1):
            j = q - R - off          # prev: in-token k = t0 - R + q
            if 0 <= j < PT:
                prev[q, j] = w[off + R]
            j = PT + q - off         # next: in-token k = t0 + PT + q
            if 0 <= j < PT:
                nxt[q, j] = w[off + R]
    first = cur.copy()
    for t in range(R):
        for j9 in range(K9):
            i_n = t + j9 - R
            if i_n < 0:
                first[-i_n, t] += w[j9]
    last = cur.copy()
    t0 = T - PT
    for tg in range(T - R, T):
        for j9 in range(K9):
            i_n = tg + j9 - R
            if i_n > T - 1:
                last[2 * (T - 1) - i_n - t0, tg - t0] += w[j9]
    return np.stack([first, cur, last]), np.stack([prev, nxt])


def kernel(x, latents_table, eps_0, eps_1, eps_2, eps_3):
    global _NC_CACHE
    x = np.asarray(x, np.float32)
    latents_table = np.asarray(latents_table, np.float32)
    eps_all = [np.ascontiguousarray(np.asarray(e, np.float32).reshape(B, T, -1))
               for e in (eps_0, eps_1, eps_2, eps_3)]

    # block-diagonal latent table [36, 18*512] (replicated on every core)
    tbd = np.zeros((NENV, NLAT), np.float32)
    for s in range(S):
        for wi in range(WS):
            col = (s * WS + wi) * L
            tbd[s * H:(s + 1) * H, col:col + L] = latents_table[s * H:(s + 1) * H,
                                                                s * WS + wi, :]

    tbd16 = tbd.astype(np.float16)
    in_maps = []
    for b in range(B):
        env = x[b, :, :NENV]
        xct = np.ascontiguousarray((env - env.mean(axis=0)).T)
        xn = np.ascontiguousarray(x[b, :, NENV:])
        bandc = np.empty((3 * N_NOISE, PT, PT), np.float32)
        bande = np.empty((N_NOISE, 2 * R, PT), np.float32)
        bmu = np.empty((T, N_NOISE), np.float32)
        for i in range(N_NOISE):
            sig_t = np.float32(max(xn[:, 3 * i + 2].mean(), 1e-3))
            k = np.arange(-R, R + 1, dtype=np.float32)
            w = np.exp(-0.5 * k * k / (sig_t * sig_t))
            w = (w / w.sum()).astype(np.float32)
            mu = xn[:, 3 * i]
            mup = np.pad(mu, (R, R), mode="reflect")
            bmu[:, i] = sum(w[j] * mup[j:j + T] for j in range(K9))
            c3, e2 = _band_blocks(w)
            bandc[3 * i:3 * i + 3] = c3
            bande[i] = e2.reshape(2 * R, PT)
        m = {"xct": xct.astype(np.float16), "tbd": tbd16, "xn": xn,
             "bandc": bandc.astype(np.float16),
             "bande": bande.astype(np.float16), "bmu": bmu}
        for i in range(N_NOISE):
            m[f"eps{i}"] = eps_all[i][b]
        in_maps.append(m)

    if _NC_CACHE is None:
        _NC_CACHE = _build_nc()
    res = run_bass_kernel_spmd(_NC_CACHE, in_maps, list(range(8)))
    global LAST_RESULTS
    LAST_RESULTS = res

    lats = np.stack([res.results[b]["lats"].reshape(T, S * WS, L) for b in range(B)])
    noises = tuple(
        np.stack([res.results[b][f"n{i}"].reshape(T, int(np.sqrt(SS[i])),
                                                  int(np.sqrt(SS[i])))
                  for b in range(B)])
        for i in range(N_NOISE)
    )
    return (lats,) + noises


# revision 30
# speedup vs baseline: 1.0053x; 1.0053x over previous
"""Trainium2 Bass kernel for nn_FixedLatentNoiseDecoder.

Strategy (pure data-parallel, 1 batch sample per NeuronCore, no collectives):

  lats branch:  lats[b,t] = (env[b,t] - mean_T env[b]) @ W_blockdiag
     - mean over T commutes with the linear map, so the input is centered on
       the host (tiny: 3 MB of x) and shipped pre-transposed [36, T].
     - W_blockdiag [36, 18*512] is built on the host from latents_table and
       replicated; the device does K=36 fp32 matmuls per 128-token tile.

  noise branch: out_i = blur(mu + sig*eps_i) along T (9-tap, per-sample sigma)
     - blur is linear: out = Band @ (sig .* eps) + (Band @ mu).
     - Band is a banded Toeplitz matrix built on the host from the 9 Gaussian
       taps (sigma derives from x only), with reflect-boundary variants for
       the first/last 128-token tile.  Per out-tile the contraction over
       128+8 input tokens is done as 3 PSUM-accumulated matmuls:
       prev-halo (K=4), current (K=128), next-halo (K=4).
     - sig is folded in by row-scaling the eps tile on the vector engine;
       Band@mu ("blurmu", derives from x only) is a host scalar per token,
       folded into the PSUM->SBUF copy as a per-partition bias on ScalarE.

All heavy I/O (86 MB of stores, 11 MB of eps loads per core) is large
contiguous DMA.
"""

import numpy as np

import concourse.bass as bass
import concourse.bacc as bacc
import concourse.mybir as mybir
import concourse.tile as tile
from concourse.bass_utils import run_bass_kernel_spmd

B, T = 8, 2048
S, H, WS, L = 3, 12, 6, 512
NENV = S * H          # 36
NLAT = S * WS * L     # 9216
R = 4
K9 = 2 * R + 1
N_NOISE = 4
SS = [16, 64, 256, 1024]   # s*s per noise level
PT = 128                   # tokens per tile
NTILE = T // PT            # 16
F32 = mybir.dt.float32
F16 = mybir.dt.float16     # matmul operand dtype: full PE rate + FWL weight loads

_NC_CACHE = None


def _build_nc():
    nc = bacc.Bacc("TRN2", target_bir_lowering=False, debug=False, num_devices=8)

    xct = nc.dram_tensor("xct", [NENV, T], F16, kind="ExternalInput")
    tbd = nc.dram_tensor("tbd", [NENV, NLAT], F16, kind="ExternalInput")
    xn = nc.dram_tensor("xn", [T, 12], F32, kind="ExternalInput")
    bandc = nc.dram_tensor("bandc", [3 * N_NOISE, PT, PT], F16, kind="ExternalInput")
    bande = nc.dram_tensor("bande", [N_NOISE, 2 * R, PT], F16, kind="ExternalInput")
    bmu = nc.dram_tensor("bmu", [T, N_NOISE], F32, kind="ExternalInput")
    eps = [nc.dram_tensor(f"eps{i}", [T, SS[i]], F32, kind="ExternalInput")
           for i in range(N_NOISE)]

    lats = nc.dram_tensor("lats", [T, NLAT], F32, kind="ExternalOutput")
    nout = [nc.dram_tensor(f"n{i}", [T, SS[i]], F32, kind="ExternalOutput")
            for i in range(N_NOISE)]

    with tile.TileContext(nc) as tc:
        with (
            tc.tile_pool(name="const", bufs=1) as cp,
            tc.tile_pool(name="lats_out", bufs=2) as lop,
            tc.tile_pool(name="eps0", bufs=8) as ep0,
            tc.tile_pool(name="eps1", bufs=8) as ep1,
            tc.tile_pool(name="eps2", bufs=8) as ep2,
            tc.tile_pool(name="eps3", bufs=8) as ep3,
            tc.tile_pool(name="epsraw", bufs=6) as epr,
            tc.tile_pool(name="nout0", bufs=2) as np0,
            tc.tile_pool(name="nout1", bufs=2) as np1,
            tc.tile_pool(name="nout2", bufs=2) as np2,
            tc.tile_pool(name="nout3", bufs=2) as np3,
            tc.tile_pool(name="tails", bufs=3) as tlp,
            tc.tile_pool(name="lpsum", bufs=3, space="PSUM") as lps,
            tc.tile_pool(name="npsum", bufs=2, space="PSUM") as nps,
        ):
            epools = [ep0, ep1, ep2, ep3]
            opools = [np0, np1, np2, np3]

            # --- constants, loaded once ---
            tb = cp.tile([NENV, NLAT], F16)
            nc.sync.dma_start(tb[:], tbd[:])
            xs = cp.tile([NENV, T], F16)
            nc.sync.dma_start(xs[:], xct[:])
            bc = cp.tile([PT, 3 * N_NOISE * PT], F16)
            for v in range(3 * N_NOISE):
                nc.gpsimd.dma_start(bc[:, v * PT:(v + 1) * PT], bandc[v])
            be = cp.tile([2 * R, N_NOISE * PT], F16)
            for v in range(N_NOISE):
                nc.gpsimd.dma_start(be[:, v * PT:(v + 1) * PT], bande[v])
            bm = cp.tile([PT, NTILE * N_NOISE], F32)
            xns = cp.tile([PT, NTILE * 12], F32)
            for n in range(NTILE):
                nc.gpsimd.dma_start(bm[:, n * N_NOISE:(n + 1) * N_NOISE],
                                  bmu[n * PT:(n + 1) * PT, :])
                nc.gpsimd.dma_start(xns[:, n * 12:(n + 1) * 12],
                                  xn[n * PT:(n + 1) * PT, :])

            # scaled-eps tiles, kept alive for the +/-1 tile halo
            seps = [[None] * NTILE for _ in range(N_NOISE)]

            def load_scale(i, n):
                s2 = SS[i]
                raw = epr.tile([PT, s2], F32, tag=f"raw{i}")
                nc.scalar.dma_start(raw[:], eps[i][n * PT:(n + 1) * PT, :])
                t_ = epools[i].tile([PT, s2], F16)
                sig = xns[:, n * 12 + 3 * i + 1: n * 12 + 3 * i + 2]
                nc.vector.tensor_scalar_mul(t_[:], raw[:], sig)
                seps[i][n] = t_

            for i in range(N_NOISE):
                for n0 in range(4):
                    load_scale(i, n0)

            def do_noise(n):
                # --- noise branch for tile n ---
                v = 0 if n == 0 else (2 if n == NTILE - 1 else 1)
                for i in range(N_NOISE):
                    s2 = SS[i]
                    hl = tlp.tile([2 * R, s2], F16, tag=f"halo{i}")
                    if n == 0 or n == NTILE - 1:
                        nc.gpsimd.memset(hl[:], 0.0)
                    if n > 0:
                        nc.gpsimd.dma_start(hl[0:R, :],
                                            seps[i][n - 1][PT - R:PT, :])
                    if n + 1 < NTILE:
                        nc.gpsimd.dma_start(hl[R:2 * R, :], seps[i][n + 1][0:R, :])
                    no = opools[i].tile([PT, s2], F32)
                    for c0 in range(0, s2, L):
                        cw = min(L, s2 - c0)
                        pq = nps.tile([PT, cw], F32)
                        nc.tensor.matmul(pq[:],
                                         bc[:, (3 * i + v) * PT:(3 * i + v + 1) * PT],
                                         seps[i][n][:, c0:c0 + cw],
                                         start=True, stop=False)
                        nc.tensor.matmul(pq[:], be[:, i * PT:(i + 1) * PT],
                                         hl[:, c0:c0 + cw],
                                         start=False, stop=True)
                        nc.scalar.activation(
                            no[:, c0:c0 + cw], pq[:],
                            mybir.ActivationFunctionType.Identity,
                            bias=bm[:, n * N_NOISE + i: n * N_NOISE + i + 1],
                            scale=1.0)
                    nc.sync.dma_start(nout[i][n * PT:(n + 1) * PT, :], no[:])

            def do_lats(n):
                # --- lats branch for tile n ---
                # two matmuls share a 2-bank PSUM tile; one wide copy drains both
                lo = lop.tile([PT, NLAT], F32)
                xt = xs[:, n * PT:(n + 1) * PT]
                for c2 in range(9):
                    pp = lps.tile([PT, 2 * L], F32)
                    for h in range(2):
                        c = 2 * c2 + h
                        nc.tensor.matmul(pp[:, h * L:(h + 1) * L], xt,
                                         tb[:, c * L:(c + 1) * L],
                                         start=True, stop=True)
                    if c2 % 3 == 2:
                        nc.scalar.copy(lo[:, c2 * 2 * L:(c2 + 1) * 2 * L], pp[:])
                    else:
                        nc.vector.tensor_copy(lo[:, c2 * 2 * L:(c2 + 1) * 2 * L],
                                              pp[:])
                parts = 4 if n < 2 else 2
                step = NLAT // parts
                for p_ in range(parts):
                    ring = nc.sync if p_ % 2 == 0 else nc.scalar
                    ring.dma_start(
                        lats[n * PT:(n + 1) * PT, p_ * step:(p_ + 1) * step],
                        lo[:, p_ * step:(p_ + 1) * step])

            # tile 0: lats first so the big stores start as early as possible
            do_lats(0)
            for n in range(NTILE):
                for i in range(N_NOISE):
                    if n + 4 < NTILE:
                        load_scale(i, n + 4)
                do_noise(n)
                if n + 1 < NTILE:
                    do_lats(n + 1)

    nc.compile()
    return nc


def _band_blocks(w):
    """9-tap blur as banded-matrix blocks.  Returns (cur[3,128,128], edge[2,4,128]).

    Block convention: out[t] = sum_k Band[k, t] * n[k]; lhsT layout is
    [K(in-token), M(out-token)].  cur variants: 0=first tile (reflect),
    1=interior, 2=last tile (reflect).  edge: 0=prev halo, 1=next halo.
    """
    cur = np.zeros((PT, PT), np.float32)
    for off in range(-R, R + 1):
        cur += w[off + R] * np.eye(PT, PT, -off, dtype=np.float32)
    prev = np.zeros((R, PT), np.float32)
    nxt = np.zeros((R, PT), np.float32)
    for q in range(R):
        for off in range(-R, R + 1):
            j = q - R - off          # prev: in-token k = t0 - R + q
            if 0 <= j < PT:
                prev[q, j] = w[off + R]
            j = PT + q - off         # next: in-token k = t0 + PT + q
            if 0 <= j < PT:
                nxt[q, j] = w[off + R]
    first = cur.copy()
    for t in range(R):
        for j9 in range(K9):
            i_n = t + j9 - R
            if i_n < 0:
                first[-i_n, t] += w[j9]
    last = cur.copy()
    t0 = T - PT
    for tg in range(T - R, T):
        for j9 in range(K9):
            i_n = tg + j9 - R
            if i_n > T - 1:
                last[2 * (T - 1) - i_n - t0, tg - t0] += w[j9]
    return np.stack([first, cur, last]), np.stack([prev, nxt])


def kernel(x, latents_table, eps_0, eps_1, eps_2, eps_3):
    global _NC_CACHE
    x = np.asarray(x, np.float32)
    latents_table = np.asarray(latents_table, np.float32)
    eps_all = [np.ascontiguousarray(np.asarray(e, np.float32).reshape(B, T, -1))
               for e in (eps_0, eps_1, eps_2, eps_3)]

    # block-diagonal latent table [36, 18*512] (replicated on every core)
    tbd = np.zeros((NENV, NLAT), np.float32)
    for s in range(S):
        for wi in range(WS):
            col = (s * WS + wi) * L
            tbd[s * H:(s + 1) * H, col:col + L] = latents_table[s * H:(s + 1) * H,
                                                                s * WS + wi, :]

    tbd16 = tbd.astype(np.float16)
    in_maps = []
    for b in range(B):
        env = x[b, :, :NENV]
        xct = np.ascontiguousarray((env - env.mean(axis=0)).T)
        xn = np.ascontiguousarray(x[b, :, NENV:])
        bandc = np.empty((3 * N_NOISE, PT, PT), np.float32)
        bande = np.empty((N_NOISE, 2 * R, PT), np.float32)
        bmu = np.empty((T, N_NOISE), np.float32)
        for i in range(N_NOISE):
            sig_t = np.float32(max(xn[:, 3 * i + 2].mean(), 1e-3))
            k = np.arange(-R, R + 1, dtype=np.float32)
            w = np.exp(-0.5 * k * k / (sig_t * sig_t))
            w = (w / w.sum()).astype(np.float32)
            mu = xn[:, 3 * i]
            mup = np.pad(mu, (R, R), mode="reflect")
            bmu[:, i] = sum(w[j] * mup[j:j + T] for j in range(K9))
            c3, e2 = _band_blocks(w)
            bandc[3 * i:3 * i + 3] = c3
            bande[i] = e2.reshape(2 * R, PT)
        m = {"xct": xct.astype(np.float16), "tbd": tbd16, "xn": xn,
             "bandc": bandc.astype(np.float16),
             "bande": bande.astype(np.float16), "bmu": bmu}
        for i in range(N_NOISE):
            m[f"eps{i}"] = eps_all[i][b]
        in_maps.append(m)

    if _NC_CACHE is None:
        _NC_CACHE = _build_nc()
    res = run_bass_kernel_spmd(_NC_CACHE, in_maps, list(range(8)))
    global LAST_RESULTS
    LAST_RESULTS = res

    lats = np.stack([res.results[b]["lats"].reshape(T, S * WS, L) for b in range(B)])
    noises = tuple(
        np.stack([res.results[b][f"n{i}"].reshape(T, int(np.sqrt(SS[i])),
                                                  int(np.sqrt(SS[i])))
                  for b in range(B)])
        for i in range(N_NOISE)
    )
    return (lats,) + noises
